# revision 12
# baseline (speedup 1.0000x reference)
"""Trainium2 Bass kernel for CodeAwareContinuousEncoder (MoE-routed heads).

Computation (per sample b):
    z = clip((values - means) / max(stds, 1e-8), -5, 5)
    hidden = gelu(z * w1 + b1)                       # (H,)
    out = hidden @ W_heads[head_idx[b]] + b_heads[head_idx[b]]   # (E,)

Strategy: expert-shard the K=100 heads across 8 NeuronCores. Host-side
routing groups samples by head (index shuffling only - the shard map);
each core receives just the weights of its ~13 heads (~3.3 MB) plus the
normalizer inputs of the samples routed to it, padded to a fixed
per-head capacity of 64 so all 8 cores run one identical SPMD program.
All arithmetic runs on-device.

Per-core dataflow (v3):
  - z on DVE in a (128, M) layout, flattened to a (1, N) row by one
    SBUF->SBUF SWDGE DMA (the only gpsimd work)
  - z broadcast across partitions by a rank-1 PE matmul ones^T x z into
    PSUM; one ACT Gelu per H-chunk reads it with per-partition
    scale=w1 / bias=b1 APs -> hidden stays transposed, no DVE broadcast
  - two segments share one (128, E) PSUM tile: a rank-2 "split-row"
    bias matmul mask(2,128)^T x [bg_even; bg_odd](2,E) initializes the
    tile (start=True) with each half's head bias - these run before the
    weights arrive and double as PE HAM warm-up - then 4 fp32 weight
    matmuls accumulate via column-group packing (tile_position 0/64)
  - PSUM->SBUF drain is a plain copy (DVE/ACT alternating), one output
    DMA per pair
  - weight DMAs split across both HWDGE rings (sync + scalar)
"""

import numpy as np
from contextlib import ExitStack

import concourse.bass as bass
import concourse.tile as tile
from concourse import bacc, mybir
from concourse.bass_utils import run_bass_kernel_spmd
from concourse.tile_rust import add_dep_helper

B, H, E, K = 4096, 256, 256, 100
NCORES = 8
P = 128
CAP = 64
F32 = mybir.dt.float32

TRACE = False
LAST_RESULT = None

_build_cache = {}


def _build(S, MCOLS):
    """SPMD per-core program. S segments of 64 slots; MCOLS z columns."""
    nc = bacc.Bacc("TRN2", target_bir_lowering=False, debug=False)
    M = MCOLS
    N = P * M
    PAIRS = (S + 1) // 2
    NBIAS = PAIRS * E

    # [p, 0:M]=v [M:2M]=m [2M:3M]=s [3M:3M+2]=b1col [3M+2:3M+4]=w1col
    vmsb = nc.dram_tensor("vmsb", [P, 3 * M + 4], F32, kind="ExternalInput").ap()
    # row j: [i*E:(i+1)*E] = b_heads[head of segment 2i+j]; [NBIAS:NBIAS+P] = mask row j
    bgm = nc.dram_tensor("bgm", [2, NBIAS + P], F32, kind="ExternalInput").ap()
    # wg[p, s*2E + c*E + e] = W_heads[head_s, c*128 + p, e]
    wg = nc.dram_tensor("wg", [P, S * 2 * E], F32, kind="ExternalInput").ap()
    # pair layout: rows [0:64] = seg 2i, rows [64:128] = seg 2i+1
    y = nc.dram_tensor("y", [PAIRS, P, E], F32, kind="ExternalOutput").ap()

    with tile.TileContext(nc) as tc, ExitStack() as ctx:
        const_pool = ctx.enter_context(tc.tile_pool(name="const", bufs=1))
        zpool = ctx.enter_context(tc.tile_pool(name="z", bufs=1))
        hpool = ctx.enter_context(tc.tile_pool(name="hidden", bufs=1))
        wpool = ctx.enter_context(tc.tile_pool(name="w", bufs=PAIRS))
        opool = ctx.enter_context(tc.tile_pool(name="osb", bufs=4))
        pp = ctx.enter_context(tc.tile_pool(name="psum", bufs=5, space="PSUM"))
        php = ctx.enter_context(tc.tile_pool(name="ph", bufs=1, space="PSUM"))

        # --- small inputs first, then weight pairs on both HWDGE rings
        t_vmsb = const_pool.tile([P, 3 * M + 4], F32)
        nc.sync.dma_start(t_vmsb[:], vmsb)
        t_bgm = const_pool.tile([2, NBIAS + P], F32)
        nc.sync.dma_start(t_bgm[:], bgm)

        wts = []
        for i in range(PAIRS):
            lo = i * 2 * 2 * E
            sz = min(2 * 2 * E, S * 2 * E - lo)
            wt = wpool.tile([P, 2 * 2 * E], F32, tag="wt")
            eng = nc.sync if i % 2 == 0 else nc.scalar
            eng.dma_start(wt[:, 0:sz], wg[:, lo : lo + sz])
            wts.append(wt)

        ones_row = const_pool.tile([1, P], F32)
        nc.vector.memset(ones_row[:], 1.0)
        # ACT table preload for Gelu (overlaps the DMA phase)
        gscr = const_pool.tile([P, 1], F32)
        nc.vector.memset(gscr[:], 0.5)
        gscr2 = const_pool.tile([P, 1], F32)
        nc.scalar.activation(gscr2[:], gscr[:], mybir.ActivationFunctionType.Gelu)

        # --- bias matmuls: initialize each pair's PSUM tile with the two
        # head biases in split rows; early (only needs bgm) = PE warm-up.
        pos = []
        bias_mms = []
        msk = t_bgm[:, NBIAS : NBIAS + P]
        for i in range(PAIRS):
            po = pp.tile([P, E], F32, tag="po")
            pos.append(po)
            mm = nc.tensor.matmul(
                po[:],
                msk,
                t_bgm[:, i * E : (i + 1) * E],
                start=True,
                stop=False,
                skip_group_check=True,
            )
            bias_mms.append(mm)
            if i >= PAIRS - 3:
                break  # last pair psums allocated later (bank budget)

        # --- z = clip((v - m) * recip(max(s, 1e-8)), -5, 5) on (128, M)
        z2d = zpool.tile([P, M], F32)
        tmp = zpool.tile([P, M], F32)
        nc.vector.tensor_sub(z2d[:], t_vmsb[:, 0:M], t_vmsb[:, M : 2 * M])
        nc.vector.tensor_scalar_max(tmp[:], t_vmsb[:, 2 * M : 3 * M], 1e-8)
        rec = zpool.tile([P, M], F32)
        nc.vector.reciprocal(rec[:], tmp[:])
        nc.vector.tensor_mul(z2d[:], z2d[:], rec[:])
        nc.vector.tensor_scalar(
            z2d[:], z2d[:], 5.0, -5.0, mybir.AluOpType.min, mybir.AluOpType.max
        )

        # --- flatten slot-major (tiny SBUF->SBUF on the scalar HWDGE
        # ring: SWDGE-on-gpsimd pays a multi-us post-DMA drain)
        zrow = zpool.tile([1, N], F32)
        nc.scalar.dma_start(zrow[:].rearrange("a (p m) -> a p m", p=P), z2d[:])
        ph = php.tile([P, N], F32)
        half = (N // 2 + 127) // 128 * 128  # multiple of 128, <= 512
        assert half <= 512
        zb_bounds = [(0, half), (half, N)]
        for lo, hi in zb_bounds:
            nc.tensor.matmul(
                ph[:, lo:hi],
                ones_row[:],
                zrow[:, lo:hi],
                start=True,
                stop=True,
            )

        # --- hidden chunks: h[c2][p, i] = gelu(z_i * w1[c2*128+p] + b1[..])
        # split along the free dim on the same boundaries as the z
        # broadcast so each gelu half only waits for its own zb matmul
        hid = []
        for c2 in range(2):
            h = hpool.tile([P, N], F32, tag=f"h{c2}")
            for lo, hi in zb_bounds:
                nc.scalar.activation(
                    h[:, lo:hi],
                    ph[:, lo:hi],
                    mybir.ActivationFunctionType.Gelu,
                    scale=t_vmsb[:, 3 * M + 2 + c2 : 3 * M + 3 + c2],
                    bias=t_vmsb[:, 3 * M + c2 : 3 * M + 1 + c2],
                )
            hid.append(h)

        # --- segment pair GEMMs, column-group packed
        for i in range(PAIRS):
            if i >= len(pos):
                po = pp.tile([P, E], F32, tag="po")
                pos.append(po)
                bias_mms.append(
                    nc.tensor.matmul(
                        po[:],
                        msk,
                        t_bgm[:, i * E : (i + 1) * E],
                        start=True,
                        stop=False,
                        skip_group_check=True,
                    )
                )
            wt = wts[i]
            po = pos[i]
            segs = [2 * i] + ([2 * i + 1] if (2 * i + 1) < S else [])
            last_mm = None
            for c2 in range(2):
                for j, s in enumerate(segs):
                    colbase = 64 * j
                    last_mm = nc.tensor.matmul(
                        po[colbase : colbase + CAP, :],
                        hid[c2][:, s * CAP : (s + 1) * CAP],
                        wt[:, (2 * j + c2) * E : (2 * j + c2 + 1) * E],
                        start=False,
                        stop=(c2 == 1),
                        tile_position=(0, colbase),
                        skip_group_check=True,
                    )
            osb = opool.tile([P, E], F32, tag="osb")
            if i % 2 == 0:
                cp = nc.vector.tensor_copy(osb[:], po[:])
            else:
                cp = nc.scalar.copy(osb[:], po[:])
            # copy reads the whole tile; deps already cover all matmuls,
            # but order explicitly after the final matmul for bank safety
            add_dep_helper(cp.ins, last_mm.ins, True, "psum drain order")
            nc.scalar.dma_start(y[i], osb[:])
    nc.compile()
    return nc


def kernel(values, means, stds, head_idx, w1, b1, W_heads, b_heads):
    global LAST_RESULT
    values = np.ascontiguousarray(values, dtype=np.float32)
    means = np.ascontiguousarray(means, dtype=np.float32)
    stds = np.ascontiguousarray(stds, dtype=np.float32)
    head_idx = np.ascontiguousarray(head_idx, dtype=np.int32)
    w1 = np.ascontiguousarray(w1, dtype=np.float32)
    b1 = np.ascontiguousarray(b1, dtype=np.float32)
    W_heads = np.ascontiguousarray(W_heads, dtype=np.float32)
    b_heads = np.ascontiguousarray(b_heads, dtype=np.float32)
    nb = values.shape[0]

    # ---- host routing: group sample indices by head, chunk to <=64 ----
    order = np.argsort(head_idx, kind="stable")
    counts = np.bincount(head_idx, minlength=K)
    bounds = np.concatenate([[0], np.cumsum(counts)])
    segments = []  # (head, idx_array)
    for k in range(K):
        idx = order[bounds[k] : bounds[k + 1]]
        for lo in range(0, len(idx), CAP):
            segments.append((k, idx[lo : lo + CAP]))
    S = -(-len(segments) // NCORES)
    while len(segments) < S * NCORES:
        segments.append((0, np.empty(0, dtype=np.int64)))
    MCOLS = -(-(S * CAP) // P)
    N = P * MCOLS
    PAIRS = (S + 1) // 2
    NBIAS = PAIRS * E

    key = (S, MCOLS)
    if key not in _build_cache:
        _build_cache[key] = _build(S, MCOLS)
    nc = _build_cache[key]

    b1col = b1.reshape(2, P).T  # (128, 2)
    w1col = w1.reshape(2, P).T  # (128, 2)
    # (K, 128, 2, E): [k, p, c, e] = W_heads[k, c*128+p, e]
    W_chunked = W_heads.reshape(K, 2, P, E).transpose(0, 2, 1, 3)

    in_maps = []
    core_segs = []
    for c in range(NCORES):
        segs = segments[c * S : (c + 1) * S]
        core_segs.append(segs)
        v_slot = np.zeros(N, np.float32)
        m_slot = np.zeros(N, np.float32)
        s_slot = np.ones(N, np.float32)
        for si, (k, idx) in enumerate(segs):
            n = len(idx)
            sl = slice(si * CAP, si * CAP + n)
            v_slot[sl] = values[idx]
            m_slot[sl] = means[idx]
            s_slot[sl] = stds[idx]
        vmsb = np.empty((P, 3 * MCOLS + 4), np.float32)
        vmsb[:, 0:MCOLS] = v_slot.reshape(P, MCOLS)
        vmsb[:, MCOLS : 2 * MCOLS] = m_slot.reshape(P, MCOLS)
        vmsb[:, 2 * MCOLS : 3 * MCOLS] = s_slot.reshape(P, MCOLS)
        vmsb[:, 3 * MCOLS : 3 * MCOLS + 2] = b1col
        vmsb[:, 3 * MCOLS + 2 : 3 * MCOLS + 4] = w1col
        heads = np.array([k for k, _ in segs], np.int64)
        bgm = np.zeros((2, NBIAS + P), np.float32)
        bg = b_heads[heads]  # (S, E)
        bgm[0, : (len(segs) + 1) // 2 * E] = bg[0::2].reshape(-1)
        bgm[1, : len(segs) // 2 * E] = bg[1::2].reshape(-1)
        bgm[0, NBIAS : NBIAS + CAP] = 1.0
        bgm[1, NBIAS + CAP : NBIAS + P] = 1.0
        # (128, S*2*E) segment-major, per-partition contiguous
        wgc = np.ascontiguousarray(
            W_chunked[heads].transpose(1, 0, 2, 3).reshape(P, S * 2 * E)
        )
        in_maps.append({"vmsb": vmsb, "bgm": bgm, "wg": wgc})

    res = run_bass_kernel_spmd(nc, in_maps, list(range(NCORES)), trace=TRACE)
    LAST_RESULT = res

    out = np.empty((nb, E), np.float32)
    for c in range(NCORES):
        yc = res.results[c]["y"]  # (PAIRS, 128, E)
        for si, (k, idx) in enumerate(core_segs[c]):
            n = len(idx)
            if n:
                out[idx] = yc[si // 2, CAP * (si % 2) : CAP * (si % 2) + n, :]
    return out


# revision 15
# speedup vs baseline: 1.0046x; 1.0046x over previous
"""Trainium2 Bass kernel for CodeAwareContinuousEncoder (MoE-routed heads).

Computation (per sample b):
    z = clip((values - means) / max(stds, 1e-8), -5, 5)
    hidden = gelu(z * w1 + b1)                       # (H,)
    out = hidden @ W_heads[head_idx[b]] + b_heads[head_idx[b]]   # (E,)

Strategy: expert-shard the K=100 heads across 8 NeuronCores. Host-side
routing groups samples by head (index shuffling only - the shard map);
each core receives just the weights of its ~13 heads (~3.3 MB) plus the
normalizer inputs of the samples routed to it, padded to a fixed
per-head capacity of 64 so all 8 cores run one identical SPMD program.
All arithmetic runs on-device.

Per-core dataflow (v3):
  - z on DVE in a (128, M) layout, flattened to a (1, N) row by one
    SBUF->SBUF SWDGE DMA (the only gpsimd work)
  - z broadcast across partitions by a rank-1 PE matmul ones^T x z into
    PSUM; one ACT Gelu per H-chunk reads it with per-partition
    scale=w1 / bias=b1 APs -> hidden stays transposed, no DVE broadcast
  - two segments share one (128, E) PSUM tile: a rank-2 "split-row"
    bias matmul mask(2,128)^T x [bg_even; bg_odd](2,E) initializes the
    tile (start=True) with each half's head bias - these run before the
    weights arrive and double as PE HAM warm-up - then 4 fp32 weight
    matmuls accumulate via column-group packing (tile_position 0/64)
  - PSUM->SBUF drain is a plain copy (DVE/ACT alternating), one output
    DMA per pair
  - weight DMAs split across both HWDGE rings (sync + scalar)
"""

import numpy as np
from contextlib import ExitStack

import concourse.bass as bass
import concourse.tile as tile
from concourse import bacc, mybir
from concourse.bass_utils import run_bass_kernel_spmd
from concourse.tile_rust import add_dep_helper

B, H, E, K = 4096, 256, 256, 100
NCORES = 8
P = 128
CAP = 64
F32 = mybir.dt.float32

TRACE = False
LAST_RESULT = None

_build_cache = {}


def _build(S, MCOLS):
    """SPMD per-core program. S segments of 64 slots; MCOLS z columns."""
    nc = bacc.Bacc("TRN2", target_bir_lowering=False, debug=False)
    M = MCOLS
    N = P * M
    PAIRS = (S + 1) // 2
    NBIAS = PAIRS * E

    # [p, 0:M]=v [M:2M]=m [2M:3M]=s [3M:3M+2]=b1col [3M+2:3M+4]=w1col
    vmsb = nc.dram_tensor("vmsb", [P, 3 * M + 4], F32, kind="ExternalInput").ap()
    # row j: [i*E:(i+1)*E] = b_heads[head of segment 2i+j]; [NBIAS:NBIAS+P] = mask row j
    bgm = nc.dram_tensor("bgm", [2, NBIAS + P], F32, kind="ExternalInput").ap()
    # wg[p, s*2E + c*E + e] = W_heads[head_s, c*128 + p, e]
    wg = nc.dram_tensor("wg", [P, S * 2 * E], F32, kind="ExternalInput").ap()
    # pair layout: rows [0:64] = seg 2i, rows [64:128] = seg 2i+1
    y = nc.dram_tensor("y", [PAIRS, P, E], F32, kind="ExternalOutput").ap()

    with tile.TileContext(nc) as tc, ExitStack() as ctx:
        const_pool = ctx.enter_context(tc.tile_pool(name="const", bufs=1))
        zpool = ctx.enter_context(tc.tile_pool(name="z", bufs=1))
        hpool = ctx.enter_context(tc.tile_pool(name="hidden", bufs=1))
        wpool = ctx.enter_context(tc.tile_pool(name="w", bufs=PAIRS))
        opool = ctx.enter_context(tc.tile_pool(name="osb", bufs=4))
        pp = ctx.enter_context(tc.tile_pool(name="psum", bufs=5, space="PSUM"))
        php = ctx.enter_context(tc.tile_pool(name="ph", bufs=1, space="PSUM"))

        # --- small inputs first, then weight pairs on both HWDGE rings
        t_vmsb = const_pool.tile([P, 3 * M + 4], F32)
        nc.sync.dma_start(t_vmsb[:], vmsb)
        t_bgm = const_pool.tile([2, NBIAS + P], F32)
        nc.sync.dma_start(t_bgm[:], bgm)

        # --- z = clip((v - m) * recip(max(s, 1e-8)), -5, 5) on (128, M);
        # the flatten dispatches FIRST on the scalar ring so zrow doesn't
        # queue behind 1.5 MB of weight transfers
        z2d = zpool.tile([P, M], F32)
        tmp = zpool.tile([P, M], F32)
        nc.vector.tensor_sub(z2d[:], t_vmsb[:, 0:M], t_vmsb[:, M : 2 * M])
        nc.vector.tensor_scalar_max(tmp[:], t_vmsb[:, 2 * M : 3 * M], 1e-8)
        rec = zpool.tile([P, M], F32)
        nc.vector.reciprocal(rec[:], tmp[:])
        nc.vector.tensor_mul(z2d[:], z2d[:], rec[:])
        nc.vector.tensor_scalar(
            z2d[:], z2d[:], 5.0, -5.0, mybir.AluOpType.min, mybir.AluOpType.max
        )
        zrow = zpool.tile([1, N], F32)
        nc.scalar.dma_start(zrow[:].rearrange("a (p m) -> a p m", p=P), z2d[:])

        wts = []
        for i in range(PAIRS):
            lo = i * 2 * 2 * E
            sz = min(2 * 2 * E, S * 2 * E - lo)
            wt = wpool.tile([P, 2 * 2 * E], F32, tag="wt")
            eng = nc.sync if i % 2 == 0 else nc.scalar
            eng.dma_start(wt[:, 0:sz], wg[:, lo : lo + sz])
            wts.append(wt)

        ones_row = const_pool.tile([1, P], F32)
        nc.vector.memset(ones_row[:], 1.0)
        # ACT table preload for Gelu (overlaps the DMA phase)
        gscr = const_pool.tile([P, 1], F32)
        nc.vector.memset(gscr[:], 0.5)
        gscr2 = const_pool.tile([P, 1], F32)
        nc.scalar.activation(gscr2[:], gscr[:], mybir.ActivationFunctionType.Gelu)

        # --- bias matmuls: initialize each pair's PSUM tile with the two
        # head biases in split rows; early (only needs bgm) = PE warm-up.
        pos = []
        bias_mms = []
        msk = t_bgm[:, NBIAS : NBIAS + P]
        for i in range(PAIRS):
            po = pp.tile([P, E], F32, tag="po")
            pos.append(po)
            mm = nc.tensor.matmul(
                po[:],
                msk,
                t_bgm[:, i * E : (i + 1) * E],
                start=True,
                stop=False,
                skip_group_check=True,
            )
            bias_mms.append(mm)
            if i >= 2:
                break  # first 3 warm the PE; rest emitted after zb/gelu

        # --- broadcast z via PE rank-1 into PSUM (right after the
        # 3 warm-up bias matmuls in the PE FIFO)
        ph = php.tile([P, N], F32)
        half = (N // 2 + 127) // 128 * 128  # multiple of 128, <= 512
        assert half <= 512
        zb_bounds = [(0, half), (half, N)]
        for lo, hi in zb_bounds:
            nc.tensor.matmul(
                ph[:, lo:hi],
                ones_row[:],
                zrow[:, lo:hi],
                start=True,
                stop=True,
            )

        # --- hidden chunks: h[c2][p, i] = gelu(z_i * w1[c2*128+p] + b1[..])
        # split along the free dim on the same boundaries as the z
        # broadcast so each gelu half only waits for its own zb matmul
        hid = []
        for c2 in range(2):
            h = hpool.tile([P, N], F32, tag=f"h{c2}")
            for lo, hi in zb_bounds:
                nc.scalar.activation(
                    h[:, lo:hi],
                    ph[:, lo:hi],
                    mybir.ActivationFunctionType.Gelu,
                    scale=t_vmsb[:, 3 * M + 2 + c2 : 3 * M + 3 + c2],
                    bias=t_vmsb[:, 3 * M + c2 : 3 * M + 1 + c2],
                )
            hid.append(h)

        # bias matmuls for pairs 3-4 (pairs 5-6 allocate inline below)
        for i in range(3, min(5, PAIRS)):
            po = pp.tile([P, E], F32, tag="po")
            pos.append(po)
            bias_mms.append(
                nc.tensor.matmul(
                    po[:],
                    msk,
                    t_bgm[:, i * E : (i + 1) * E],
                    start=True,
                    stop=False,
                    skip_group_check=True,
                )
            )

        # --- segment pair GEMMs, column-group packed
        for i in range(PAIRS):
            if i >= len(pos):
                po = pp.tile([P, E], F32, tag="po")
                pos.append(po)
                bias_mms.append(
                    nc.tensor.matmul(
                        po[:],
                        msk,
                        t_bgm[:, i * E : (i + 1) * E],
                        start=True,
                        stop=False,
                        skip_group_check=True,
                    )
                )
            wt = wts[i]
            po = pos[i]
            segs = [2 * i] + ([2 * i + 1] if (2 * i + 1) < S else [])
            last_mm = None
            for c2 in range(2):
                for j, s in enumerate(segs):
                    colbase = 64 * j
                    last_mm = nc.tensor.matmul(
                        po[colbase : colbase + CAP, :],
                        hid[c2][:, s * CAP : (s + 1) * CAP],
                        wt[:, (2 * j + c2) * E : (2 * j + c2 + 1) * E],
                        start=False,
                        stop=(c2 == 1),
                        tile_position=(0, colbase),
                        skip_group_check=True,
                    )
            osb = opool.tile([P, E], F32, tag="osb")
            if i % 2 == 0:
                cp = nc.vector.tensor_copy(osb[:], po[:])
            else:
                cp = nc.scalar.copy(osb[:], po[:])
            # copy reads the whole tile; deps already cover all matmuls,
            # but order explicitly after the final matmul for bank safety
            add_dep_helper(cp.ins, last_mm.ins, True, "psum drain order")
            nc.scalar.dma_start(y[i], osb[:])
    nc.compile()
    return nc


def kernel(values, means, stds, head_idx, w1, b1, W_heads, b_heads):
    global LAST_RESULT
    values = np.ascontiguousarray(values, dtype=np.float32)
    means = np.ascontiguousarray(means, dtype=np.float32)
    stds = np.ascontiguousarray(stds, dtype=np.float32)
    head_idx = np.ascontiguousarray(head_idx, dtype=np.int32)
    w1 = np.ascontiguousarray(w1, dtype=np.float32)
    b1 = np.ascontiguousarray(b1, dtype=np.float32)
    W_heads = np.ascontiguousarray(W_heads, dtype=np.float32)
    b_heads = np.ascontiguousarray(b_heads, dtype=np.float32)
    nb = values.shape[0]

    # ---- host routing: group sample indices by head, chunk to <=64 ----
    order = np.argsort(head_idx, kind="stable")
    counts = np.bincount(head_idx, minlength=K)
    bounds = np.concatenate([[0], np.cumsum(counts)])
    segments = []  # (head, idx_array)
    for k in range(K):
        idx = order[bounds[k] : bounds[k + 1]]
        for lo in range(0, len(idx), CAP):
            segments.append((k, idx[lo : lo + CAP]))
    S = -(-len(segments) // NCORES)
    while len(segments) < S * NCORES:
        segments.append((0, np.empty(0, dtype=np.int64)))
    MCOLS = -(-(S * CAP) // P)
    N = P * MCOLS
    PAIRS = (S + 1) // 2
    NBIAS = PAIRS * E

    key = (S, MCOLS)
    if key not in _build_cache:
        _build_cache[key] = _build(S, MCOLS)
    nc = _build_cache[key]

    b1col = b1.reshape(2, P).T  # (128, 2)
    w1col = w1.reshape(2, P).T  # (128, 2)
    # (K, 128, 2, E): [k, p, c, e] = W_heads[k, c*128+p, e]
    W_chunked = W_heads.reshape(K, 2, P, E).transpose(0, 2, 1, 3)

    in_maps = []
    core_segs = []
    for c in range(NCORES):
        segs = segments[c * S : (c + 1) * S]
        core_segs.append(segs)
        v_slot = np.zeros(N, np.float32)
        m_slot = np.zeros(N, np.float32)
        s_slot = np.ones(N, np.float32)
        for si, (k, idx) in enumerate(segs):
            n = len(idx)
            sl = slice(si * CAP, si * CAP + n)
            v_slot[sl] = values[idx]
            m_slot[sl] = means[idx]
            s_slot[sl] = stds[idx]
        vmsb = np.empty((P, 3 * MCOLS + 4), np.float32)
        vmsb[:, 0:MCOLS] = v_slot.reshape(P, MCOLS)
        vmsb[:, MCOLS : 2 * MCOLS] = m_slot.reshape(P, MCOLS)
        vmsb[:, 2 * MCOLS : 3 * MCOLS] = s_slot.reshape(P, MCOLS)
        vmsb[:, 3 * MCOLS : 3 * MCOLS + 2] = b1col
        vmsb[:, 3 * MCOLS + 2 : 3 * MCOLS + 4] = w1col
        heads = np.array([k for k, _ in segs], np.int64)
        bgm = np.zeros((2, NBIAS + P), np.float32)
        bg = b_heads[heads]  # (S, E)
        bgm[0, : (len(segs) + 1) // 2 * E] = bg[0::2].reshape(-1)
        bgm[1, : len(segs) // 2 * E] = bg[1::2].reshape(-1)
        bgm[0, NBIAS : NBIAS + CAP] = 1.0
        bgm[1, NBIAS + CAP : NBIAS + P] = 1.0
        # (128, S*2*E) segment-major, per-partition contiguous
        wgc = np.ascontiguousarray(
            W_chunked[heads].transpose(1, 0, 2, 3).reshape(P, S * 2 * E)
        )
        in_maps.append({"vmsb": vmsb, "bgm": bgm, "wg": wgc})

    res = run_bass_kernel_spmd(nc, in_maps, list(range(NCORES)), trace=TRACE)
    LAST_RESULT = res

    out = np.empty((nb, E), np.float32)
    for c in range(NCORES):
        yc = res.results[c]["y"]  # (PAIRS, 128, E)
        for si, (k, idx) in enumerate(core_segs[c]):
            n = len(idx)
            if n:
                out[idx] = yc[si // 2, CAP * (si % 2) : CAP * (si % 2) + n, :]
    return out
